# revision 1
# baseline (speedup 1.0000x reference)
"""Trainium2 Bass kernel for nn_CrossAttention (B=8, N=4096, S=512, D=512, H=8).

Sharding: data-parallel over batch — each of the 8 NeuronCores computes the
full cross-attention for one batch element. No collectives needed.

Per-core dataflow (all activations kept feature-major so no on-chip
transposes are ever required):
  - host pre-transposes x[b] -> xT [D, N] and context[b] -> ctxT [D, S]
  - qT[o, n]  = sum_i wqT[i, o] * xT[i, n]          (PE)
  - kT[dk, s] = sum_i wkT[i, dk] * ctxT[i, s]       (PE)
  - v[s, dv]  = sum_i ctxT[i, s] * wvT[i, dv]       (PE, token-major v)
    v is stored interleaved with a ones column per head: vext[s, h, 0:64]=v,
    vext[s, h, 64]=1 so the attention matmul also produces the softmax
    denominator for free (M=65).
  - scoresT[s, n] = kT_h.T @ qT_h per head          (PE, K=64, heads paired
    into PE row groups 0-63 / 64-127 for concurrency)
  - e = exp(SCALE*scoresT + amask_bias)             (ACT; mask folded into a
    per-partition bias so masked rows give exp(-30000)=0; no max-subtraction
    needed since |SCALE*scores| is O(1) for this problem scale)
  - OtildeT'[{d,den}, n] = vext_h.T @ e_h           (PE, K=128, M=65)
  - rden = 1/den  (DVE), broadcast across partitions via a DRAM bounce
  - OT = OtildeT * R                                (DVE)
  - y[n, o] = sum_c OT[c, n-slice].T @ wpT[c, o]    (PE, token-major output,
    so the DMA store to DRAM is contiguous)

Matmul inputs are kept in MMDT (float32r or bfloat16, env KMMDT to override);
accumulation is always fp32 in PSUM and the softmax/normalization runs fp32.
"""

import os

import numpy as np

try:
    import concourse.bass as bass
except ImportError:
    import sys

    sys.path.insert(0, "/opt/trn_rl_repo")
    import concourse.bass as bass

from contextlib import ExitStack

import concourse.mybir as mybir
import concourse.tile as tile
from concourse.bass import ts

B, N, S, D, H = 8, 4096, 512, 512, 8
HD = D // H  # 64
SCALE = HD**-0.5
P = 128
IC = D // P  # 4 chunks of the contraction/feature dims
SC = S // P  # 4 chunks of the context length
NT = 512  # queries per outer tile
NTILES = N // NT  # 8
NSUB = NT // P  # 4
MASK_NEG = -30000.0

f32 = mybir.dt.float32

MMDT_NAME = os.environ.get("KMMDT", "float32r")


def _np_mm(mmdt):
    return np.dtype(mybir.dt.np(mmdt))


def _split_multi_waits(nc: bass.Bass) -> None:
    """This walrus toolchain accepts at most ONE sync-wait per instruction
    ("Too many sync wait commands" in setupSyncWait, seen for MM/LW, NoOp,
    and DMA structs alike). Hoist all but the last wait of any instruction
    onto a chain of same-engine InstNoOps spliced immediately before it —
    same program position, so synchronization semantics are unchanged."""
    eng_map = {
        mybir.EngineType.PE: lambda: nc.tensor,
        mybir.EngineType.Activation: lambda: nc.scalar,
        mybir.EngineType.DVE: lambda: nc.vector,
        mybir.EngineType.Pool: lambda: nc.gpsimd,
        mybir.EngineType.SP: lambda: nc.sync,
    }
    for fn in nc.m.functions:
        blocks = fn.blocks
        for bb in blocks:
            insts = list(bb.instructions)
            out = []
            changed = False
            for inst in insts:
                si = inst.sync_info
                if (
                    si is not None
                    and len(si.on_wait) > 1
                    and inst.engine in eng_map
                ):
                    waits = list(si.on_wait)
                    for w in waits[:-1]:  # one nop per excess wait
                        nop = eng_map[inst.engine]().nop(nofuse=True).ins
                        # the nop was appended to whatever block is current;
                        # strip it from there before splicing it in place
                        for bb2 in blocks:
                            lst = list(bb2.instructions)
                            if any(x.name == nop.name for x in lst):
                                bb2.instructions = [
                                    x for x in lst if x.name != nop.name
                                ]
                                if bb2 is bb:
                                    insts = [
                                        x for x in insts if x.name != nop.name
                                    ]
                        nop.sync_info = mybir.SyncInfo(
                            on_wait=[w], on_update=[]
                        )
                        out.append(nop)
                    inst.sync_info = mybir.SyncInfo(
                        on_wait=waits[-1:], on_update=list(si.on_update)
                    )
                    changed = True
                out.append(inst)
            if changed:
                bb.instructions = out


def _build_nc(mmdt_name: str, has_bq, has_bk, has_bv, has_bp) -> bass.Bass:
    mmdt = getattr(mybir.dt, mmdt_name)
    nc = bass.Bass()

    xT = nc.dram_tensor("xT", [D, N], mmdt, kind="ExternalInput")
    ctxT = nc.dram_tensor("ctxT", [D, S], mmdt, kind="ExternalInput")
    wqT = nc.dram_tensor("wqT", [D, D], mmdt, kind="ExternalInput")
    wkT = nc.dram_tensor("wkT", [D, D], mmdt, kind="ExternalInput")
    wvT = nc.dram_tensor("wvT", [D, D], mmdt, kind="ExternalInput")
    wpT = nc.dram_tensor("wpT", [D, D], mmdt, kind="ExternalInput")
    bq = nc.dram_tensor("bq", [D, 1], f32, kind="ExternalInput")
    bk = nc.dram_tensor("bk", [D, 1], f32, kind="ExternalInput")
    bv = nc.dram_tensor("bv", [1, D], mmdt, kind="ExternalInput")
    bp = nc.dram_tensor("bp", [1, D], mmdt, kind="ExternalInput")
    amask = nc.dram_tensor("amask", [S, 1], f32, kind="ExternalInput")
    y = nc.dram_tensor("y", [N, D], f32, kind="ExternalOutput")

    rden_dram = nc.dram_tensor("rden_scratch", [NTILES, H, NT], f32)

    ch = lambda dram: dram.rearrange("(c p) o -> p c o", p=P)  # [P, IC, D]

    with tile.TileContext(nc) as tc, ExitStack() as ctx:
        const = ctx.enter_context(tc.tile_pool(name="const", bufs=1))
        work = ctx.enter_context(tc.tile_pool(name="work", bufs=2))
        epool = ctx.enter_context(tc.tile_pool(name="epool", bufs=12))
        ypool = ctx.enter_context(tc.tile_pool(name="ypool", bufs=4))
        psum = ctx.enter_context(tc.tile_pool(name="psum", bufs=1, space="PSUM"))

        # ---- persistent tiles -------------------------------------------
        wq_t = const.tile([P, IC, D], mmdt)
        wk_t = const.tile([P, IC, D], mmdt)
        wv_t = const.tile([P, IC, D], mmdt)
        wp_t = const.tile([P, IC, D], mmdt)
        ctx_t = const.tile([P, IC, S], mmdt)
        amask_t = const.tile([P, SC, 1], f32)
        nc.sync.dma_start(wq_t[:], ch(wqT))
        nc.sync.dma_start(wk_t[:], ch(wkT))
        nc.sync.dma_start(wv_t[:], ch(wvT))
        nc.sync.dma_start(wp_t[:], ch(wpT))
        nc.sync.dma_start(ctx_t[:], ch(ctxT))
        nc.sync.dma_start(amask_t[:], amask.rearrange("(c p) o -> p c o", p=P))

        if has_bq:
            bq_t = const.tile([P, IC, 1], f32)
            nc.sync.dma_start(bq_t[:], bq.rearrange("(c p) o -> p c o", p=P))
        if has_bk:
            bk_t = const.tile([P, IC, 1], f32)
            nc.sync.dma_start(bk_t[:], bk.rearrange("(c p) o -> p c o", p=P))
        if has_bv or has_bp:
            ones1_t = const.tile([1, P], mmdt)
            nc.vector.memset(ones1_t[:], 1.0)
        if has_bv:
            bv_t = const.tile([1, D], mmdt)
            nc.sync.dma_start(bv_t[:], bv[:])
        if has_bp:
            bp_t = const.tile([1, D], mmdt)
            nc.sync.dma_start(bp_t[:], bp[:])

        kT_t = const.tile([P, IC, S], mmdt)  # feature-major keys
        vext_t = const.tile([P, SC, H, HD + 1], mmdt)  # token-major v + ones col

        # ---- kv projections (once per core) -----------------------------
        ones_cast = f32 if mmdt_name == "float32r" else mmdt
        for sc in range(SC):
            for h in range(H):
                nc.vector.memset(vext_t[:, sc, h, HD : HD + 1].bitcast(ones_cast), 1.0)

        for kc in range(IC):  # dk chunks -> kT
            ps = psum.tile([P, S], f32, tag="ps_q", bufs=2)
            for i in range(IC):
                nc.tensor.matmul(
                    ps[:],
                    wk_t[:, i, ts(kc, P)],
                    ctx_t[:, i, :],
                    start=(i == 0),
                    stop=(i == IC - 1),
                )
            if has_bk:
                nc.vector.tensor_scalar_add(kT_t[:, kc, :], ps[:], bk_t[:, kc, :])
            else:
                nc.vector.tensor_copy(kT_t[:, kc, :], ps[:])

        for sc in range(SC):  # s chunks -> v (token-major)
            ps = psum.tile([P, D], f32, tag="ps_o", bufs=2)
            for i in range(IC):
                nc.tensor.matmul(
                    ps[:],
                    ctx_t[:, i, ts(sc, P)],
                    wv_t[:, i, :],
                    start=(i == 0),
                    stop=(i == IC - 1 and not has_bv),
                )
            if has_bv:
                nc.tensor.matmul(ps[:], ones1_t[:], bv_t[:], start=False, stop=True)
            for h in range(H):
                nc.vector.tensor_copy(
                    vext_t[:, sc, h, 0:HD], ps[:, h * HD : (h + 1) * HD]
                )

        # ---- main loop over query tiles ---------------------------------
        for t in range(NTILES):
            xT_t = work.tile([P, IC, NT], mmdt, tag="xT")
            nc.sync.dma_start(
                xT_t[:], xT[:, ts(t, NT)].rearrange("(c p) n -> p c n", p=P)
            )

            # qT for this tile (feature-major)
            qT_t = work.tile([P, IC, NT], mmdt, tag="qT")
            for oc in range(IC):
                ps = psum.tile([P, NT], f32, tag="ps_q", bufs=2)
                for i in range(IC):
                    nc.tensor.matmul(
                        ps[:],
                        wq_t[:, i, ts(oc, P)],
                        xT_t[:, i, :],
                        start=(i == 0),
                        stop=(i == IC - 1),
                    )
                if has_bq:
                    nc.vector.tensor_scalar_add(qT_t[:, oc, :], ps[:], bq_t[:, oc, :])
                else:
                    nc.vector.tensor_copy(qT_t[:, oc, :], ps[:])

            oexts = {}
            for c in range(IC):  # head pair (2c, 2c+1); kT/qT chunk c
                es = {0: [], 1: []}
                for sc in range(SC):
                    for par in (0, 1):  # PE row groups 0-63 / 64-127
                        pslc = slice(par * HD, (par + 1) * HD)
                        ps_s = psum.tile([P, NT], f32, tag="ps_s", bufs=3)
                        nc.tensor.matmul(
                            ps_s[:],
                            kT_t[pslc, c, ts(sc, P)],
                            qT_t[pslc, c, :],
                            start=True,
                            stop=True,
                        )
                        e = epool.tile([P, NT], mmdt, tag="e")
                        nc.scalar.activation(
                            e[:],
                            ps_s[:],
                            mybir.ActivationFunctionType.Exp,
                            bias=amask_t[:, sc, :],
                            scale=SCALE,
                        )
                        es[par].append(e)

                for par in (0, 1):
                    h = 2 * c + par
                    ps_o = psum.tile([P, NT], f32, tag="ps_o", bufs=2)
                    for sc in range(SC):
                        nc.tensor.matmul(
                            ps_o[0 : HD + 1, :],
                            vext_t[:, sc, h, :],
                            es[par][sc][:],
                            start=(sc == 0),
                            stop=(sc == SC - 1),
                        )
                    # Otilde' (rows 0-63 = unnormalized out, row 64 = denom)
                    oe = epool.tile([HD + 1, NT], f32, tag="oext")
                    nc.vector.tensor_copy(oe[:], ps_o[0 : HD + 1, :])
                    oexts[h] = oe
                    # denominator row straight to DRAM for the broadcast bounce
                    nc.sync.dma_start(rden_dram[t, h : h + 1], oe[HD : HD + 1, :])

            # fetch denominators broadcast across 64 partitions per head,
            # reciprocal after broadcast, then normalize Otilde -> OT.
            # Compute ops need all operands at the same start partition, so the
            # odd head of each pair is staged into partitions 64:128 via an
            # SBUF->SBUF DMA (DMA has no partition-alignment restriction).
            ot_t = work.tile([P, IC, NT], mmdt, tag="ot")
            stag_t = work.tile([P, IC, NT], f32, tag="stag")
            den_t = work.tile([P, IC, NT], f32, tag="den")
            for c in range(IC):
                for par in (0, 1):
                    nc.sync.dma_start(
                        den_t[par * HD : (par + 1) * HD, c, :],
                        rden_dram[t, 2 * c + par : 2 * c + par + 1].to_broadcast(
                            (HD, NT)
                        ),
                    )
                nc.vector.reciprocal(den_t[:, c, :], den_t[:, c, :])
                nc.vector.tensor_mul(
                    ot_t[0:HD, c, :], oexts[2 * c][0:HD, :], den_t[0:HD, c, :]
                )
                nc.sync.dma_start(stag_t[HD:P, c, :], oexts[2 * c + 1][0:HD, :])
                nc.vector.tensor_mul(
                    ot_t[HD:P, c, :], stag_t[HD:P, c, :], den_t[HD:P, c, :]
                )

            # output projection, token-major
            for ns in range(NSUB):
                ps_y = psum.tile([P, D], f32, tag="ps_y", bufs=1)
                for c in range(IC):
                    nc.tensor.matmul(
                        ps_y[:],
                        ot_t[:, c, ts(ns, P)],
                        wp_t[:, c, :],
                        start=(c == 0),
                        stop=(c == IC - 1 and not has_bp),
                    )
                if has_bp:
                    nc.tensor.matmul(ps_y[:], ones1_t[:], bp_t[:], start=False, stop=True)
                y_t = ypool.tile([P, D], f32, tag="y")
                nc.vector.tensor_copy(y_t[:], ps_y[:])
                nc.sync.dma_start(y[t * NT + ns * P : t * NT + (ns + 1) * P, :], y_t[:])

    _split_multi_waits(nc)
    return nc


_NC_CACHE: dict = {}


def _get_nc(flags):
    if flags not in _NC_CACHE:
        _NC_CACHE[flags] = _build_nc(*flags)
    return _NC_CACHE[flags]


def _prep_in_maps(x, context, context_mask, wq, bq, wkv, bkv, wp, bp, mmdt_name=None):
    if mmdt_name is None:
        mmdt_name = MMDT_NAME
    np_mm = _np_mm(getattr(mybir.dt, mmdt_name))
    cvt = lambda a: np.ascontiguousarray(a).astype(np_mm, copy=False)
    wqT = cvt(wq.T)
    wkT = cvt(wkv[:D].T)
    wvT = cvt(wkv[D:].T)
    wpT = cvt(wp.T)
    bq_c = np.ascontiguousarray(bq.reshape(D, 1), dtype=np.float32)
    bk_c = np.ascontiguousarray(bkv[:D].reshape(D, 1), dtype=np.float32)
    bv_r = cvt(bkv[D:].reshape(1, D))
    bp_r = cvt(bp.reshape(1, D))
    flags = (
        mmdt_name,
        bool(np.any(bq != 0)),
        bool(np.any(bkv[:D] != 0)),
        bool(np.any(bkv[D:] != 0)),
        bool(np.any(bp != 0)),
    )
    in_maps = []
    for b in range(B):
        amask = np.where(context_mask[b], np.float32(MASK_NEG), np.float32(0.0))
        in_maps.append(
            {
                "xT": cvt(x[b].T),
                "ctxT": cvt(context[b].T),
                "wqT": wqT,
                "wkT": wkT,
                "wvT": wvT,
                "wpT": wpT,
                "bq": bq_c,
                "bk": bk_c,
                "bv": bv_r,
                "bp": bp_r,
                "amask": amask.astype(np.float32).reshape(S, 1),
            }
        )
    return in_maps, flags


def kernel(x, context, context_mask, wq, bq, wkv, bkv, wp, bp):
    from concourse.bass_utils import run_bass_kernel_spmd

    in_maps, flags = _prep_in_maps(
        x, context, context_mask, wq, bq, wkv, bkv, wp, bp
    )
    nc = _get_nc(flags)
    res = run_bass_kernel_spmd(nc, in_maps, list(range(B)))
    return np.stack([np.asarray(res.results[b]["y"]) for b in range(B)], axis=0)



# revision 22
# speedup vs baseline: 1.4187x; 1.4187x over previous
"""Trainium2 Bass kernel for nn_CrossAttention (B=8, N=4096, S=512, D=512, H=8).

Sharding: data-parallel over batch — each of the 8 NeuronCores computes the
full cross-attention for one batch element. No collectives needed.

Per-core dataflow (all activations kept feature-major so no on-chip
transposes are ever required):
  - host pre-transposes x[b] -> xT [D, N] and context[b] -> ctxT [D, S]
  - qT[o, n]  = sum_i wqT[i, o] * xT[i, n]          (PE)
  - kT[dk, s] = sum_i wkT[i, dk] * ctxT[i, s]       (PE)
  - v[s, dv]  = sum_i ctxT[i, s] * wvT[i, dv]       (PE, token-major v)
    v is stored interleaved with a ones column per head: vext[s, h, 0:64]=v,
    vext[s, h, 64]=1 so the attention matmul also produces the softmax
    denominator for free (M=65).
  - scoresT[s, n] = kT_h.T @ qT_h per head          (PE, K=64, heads paired
    into PE row groups 0-63 / 64-127 for concurrency)
  - e = exp(SCALE*scoresT + amask_bias)             (ACT; mask folded into a
    per-partition bias so masked rows give exp(-30000)=0; no max-subtraction
    needed since |SCALE*scores| is O(1) for this problem scale)
  - OtildeT'[{d,den}, n] = vext_h.T @ e_h           (PE, K=128, M=65)
  - rden = 1/den  (DVE), broadcast across partitions via a DRAM bounce
  - OT = OtildeT * R                                (DVE)
  - y[n, o] = sum_c OT[c, n-slice].T @ wpT[c, o]    (PE, token-major output,
    so the DMA store to DRAM is contiguous)

Matmul inputs are kept in MMDT (float32r or bfloat16, env KMMDT to override);
accumulation is always fp32 in PSUM and the softmax/normalization runs fp32.
"""

import os

import numpy as np

try:
    import concourse.bass as bass
except ImportError:
    import sys

    sys.path.insert(0, "/opt/trn_rl_repo")
    import concourse.bass as bass

from contextlib import ExitStack

import concourse.mybir as mybir
import concourse.tile as tile
from concourse.bass import ts

B, N, S, D, H = 8, 4096, 512, 512, 8
HD = D // H  # 64
SCALE = HD**-0.5
P = 128
IC = D // P  # 4 chunks of the contraction/feature dims
SC = S // P  # 4 chunks of the context length
NT = 512  # queries per outer tile
NTILES = N // NT  # 8
NSUB = NT // P  # 4
MASK_NEG = -30000.0

f32 = mybir.dt.float32

MMDT_NAME = os.environ.get("KMMDT", "bfloat16")


def _np_mm(mmdt):
    return np.dtype(mybir.dt.np(mmdt))


def _split_multi_waits(nc: bass.Bass) -> None:
    """This walrus toolchain accepts at most ONE sync-wait per instruction
    ("Too many sync wait commands" in setupSyncWait, seen for MM/LW, NoOp,
    and DMA structs alike). Hoist all but the last wait of any instruction
    onto a chain of same-engine InstNoOps spliced immediately before it —
    same program position, so synchronization semantics are unchanged."""
    eng_map = {
        mybir.EngineType.PE: lambda: nc.tensor,
        mybir.EngineType.Activation: lambda: nc.scalar,
        mybir.EngineType.DVE: lambda: nc.vector,
        mybir.EngineType.Pool: lambda: nc.gpsimd,
        mybir.EngineType.SP: lambda: nc.sync,
    }
    for fn in nc.m.functions:
        blocks = fn.blocks
        for bb in blocks:
            insts = list(bb.instructions)
            out = []
            changed = False
            for inst in insts:
                si = inst.sync_info
                if (
                    si is not None
                    and len(si.on_wait) > 1
                    and inst.engine in eng_map
                ):
                    waits = list(si.on_wait)
                    for w in waits[:-1]:  # one nop per excess wait
                        nop = eng_map[inst.engine]().nop(nofuse=True).ins
                        # the nop was appended to whatever block is current;
                        # strip it from there before splicing it in place
                        for bb2 in blocks:
                            lst = list(bb2.instructions)
                            if any(x.name == nop.name for x in lst):
                                bb2.instructions = [
                                    x for x in lst if x.name != nop.name
                                ]
                                if bb2 is bb:
                                    insts = [
                                        x for x in insts if x.name != nop.name
                                    ]
                        nop.sync_info = mybir.SyncInfo(
                            on_wait=[w], on_update=[]
                        )
                        out.append(nop)
                    inst.sync_info = mybir.SyncInfo(
                        on_wait=waits[-1:], on_update=list(si.on_update)
                    )
                    changed = True
                out.append(inst)
            if changed:
                bb.instructions = out


def _build_nc(mmdt_name: str, has_bq, has_bk, has_bv, has_bp) -> bass.Bass:
    mmdt = getattr(mybir.dt, mmdt_name)
    nc = bass.Bass()

    xT = nc.dram_tensor("xT", [D, N], mmdt, kind="ExternalInput")
    ctxT = nc.dram_tensor("ctxT", [D, S], mmdt, kind="ExternalInput")
    wqT = nc.dram_tensor("wqT", [D, D], mmdt, kind="ExternalInput")
    wkT = nc.dram_tensor("wkT", [D, D], mmdt, kind="ExternalInput")
    wvT = nc.dram_tensor("wvT", [D, D], mmdt, kind="ExternalInput")
    wpT = nc.dram_tensor("wpT", [D, D], mmdt, kind="ExternalInput")
    bq = nc.dram_tensor("bq", [D, 1], f32, kind="ExternalInput")
    bk = nc.dram_tensor("bk", [D, 1], f32, kind="ExternalInput")
    bv = nc.dram_tensor("bv", [1, D], mmdt, kind="ExternalInput")
    bp = nc.dram_tensor("bp", [1, D], mmdt, kind="ExternalInput")
    amask = nc.dram_tensor("amask", [S, 1], f32, kind="ExternalInput")
    y = nc.dram_tensor("y", [N, D], f32, kind="ExternalOutput")

    # reciprocal denominators, head-permuted: [tile, par, c, n] where head
    # h = 2c+par lives at [par, c] -- so one broadcast DMA serves each par.
    rden_dram = nc.dram_tensor("rden_scratch", [NTILES, 2, IC, NT], f32)

    ch = lambda dram: dram.rearrange("(c p) o -> p c o", p=P)  # [P, IC, D]

    with tile.TileContext(nc) as tc, ExitStack() as ctx:
        const = ctx.enter_context(tc.tile_pool(name="const", bufs=1))
        work = ctx.enter_context(tc.tile_pool(name="work", bufs=2))
        epool = ctx.enter_context(tc.tile_pool(name="epool", bufs=12))
        ypool = ctx.enter_context(tc.tile_pool(name="ypool", bufs=4))
        psum = ctx.enter_context(tc.tile_pool(name="psum", bufs=1, space="PSUM"))

        # ---- persistent tiles -------------------------------------------
        wq_t = const.tile([P, IC, D], mmdt)
        wk_t = const.tile([P, IC, D], mmdt)
        wv_t = const.tile([P, IC, D], mmdt)
        wp_t = const.tile([P, IC, D], mmdt)
        ctx_t = const.tile([P, IC, S], mmdt)
        amask_t = const.tile([P, SC, 1], f32)
        nc.sync.dma_start(wq_t[:], ch(wqT))
        nc.sync.dma_start(wk_t[:], ch(wkT))
        nc.sync.dma_start(wv_t[:], ch(wvT))
        nc.sync.dma_start(wp_t[:], ch(wpT))
        nc.sync.dma_start(ctx_t[:], ch(ctxT))
        nc.sync.dma_start(amask_t[:], amask.rearrange("(c p) o -> p c o", p=P))

        if has_bq:
            bq_t = const.tile([P, IC, 1], f32)
            nc.sync.dma_start(bq_t[:], bq.rearrange("(c p) o -> p c o", p=P))
        if has_bk:
            bk_t = const.tile([P, IC, 1], f32)
            nc.sync.dma_start(bk_t[:], bk.rearrange("(c p) o -> p c o", p=P))
        if has_bv or has_bp:
            ones1_t = const.tile([1, P], mmdt)
            nc.vector.memset(ones1_t[:], 1.0)
        if has_bv:
            bv_t = const.tile([1, D], mmdt)
            nc.sync.dma_start(bv_t[:], bv[:])
        if has_bp:
            bp_t = const.tile([1, D], mmdt)
            nc.sync.dma_start(bp_t[:], bp[:])

        kT_t = const.tile([P, IC, S], mmdt)  # feature-major keys
        vext_t = const.tile([P, SC, H, HD + 1], mmdt)  # token-major v + ones col

        # ---- kv projections (once per core) -----------------------------
        ones_cast = f32 if mmdt_name == "float32r" else mmdt
        for sc in range(SC):
            for h in range(H):
                nc.vector.memset(vext_t[:, sc, h, HD : HD + 1].bitcast(ones_cast), 1.0)

        for kc in range(IC):  # dk chunks -> kT
            ps = psum.tile([P, S], f32, tag="ps_q", bufs=2)
            for i in range(IC):
                nc.tensor.matmul(
                    ps[:],
                    wk_t[:, i, ts(kc, P)],
                    ctx_t[:, i, :],
                    start=(i == 0),
                    stop=(i == IC - 1),
                )
            if has_bk:
                nc.vector.tensor_scalar_add(kT_t[:, kc, :], ps[:], bk_t[:, kc, :])
            else:
                nc.vector.tensor_copy(kT_t[:, kc, :], ps[:])

        for sc in range(SC):  # s chunks -> v (token-major)
            ps = psum.tile([P, D], f32, tag="ps_o", bufs=2)
            for i in range(IC):
                nc.tensor.matmul(
                    ps[:],
                    ctx_t[:, i, ts(sc, P)],
                    wv_t[:, i, :],
                    start=(i == 0),
                    stop=(i == IC - 1 and not has_bv),
                )
            if has_bv:
                nc.tensor.matmul(ps[:], ones1_t[:], bv_t[:], start=False, stop=True)
            for h in range(H):
                nc.vector.tensor_copy(
                    vext_t[:, sc, h, 0:HD], ps[:, h * HD : (h + 1) * HD]
                )

        # ---- main loop over query tiles ---------------------------------
        # Software-pipelined: tile t's output projection is emitted AFTER
        # tile t+1's attention matmuls so the PE never sits behind tile t's
        # denominator round-trip (engine queues are in-order; a y-proj
        # waiting on the normalize would head-of-line-block the next tile).
        def emit_yproj(t, ot_t):
            for ns in range(NSUB):
                ps_y = psum.tile([P, D], f32, tag="ps_o", bufs=2)
                for c in range(IC):
                    nc.tensor.matmul(
                        ps_y[:],
                        ot_t[:, c, ts(ns, P)],
                        wp_t[:, c, :],
                        start=(c == 0),
                        stop=(c == IC - 1 and not has_bp),
                    )
                if has_bp:
                    nc.tensor.matmul(ps_y[:], ones1_t[:], bp_t[:], start=False, stop=True)
                y_t = ypool.tile([P, D], f32, tag="y")
                nc.vector.tensor_copy(y_t[:], ps_y[:])
                nc.sync.dma_start(y[t * NT + ns * P : t * NT + (ns + 1) * P, :], y_t[:])

        prev = None  # (t, ot_t) awaiting output projection
        for t in range(NTILES):
            xT_t = work.tile([P, IC, NT], mmdt, tag="xT")
            nc.sync.dma_start(
                xT_t[:], xT[:, ts(t, NT)].rearrange("(c p) n -> p c n", p=P)
            )

            # qT for this tile (feature-major)
            qT_t = work.tile([P, IC, NT], mmdt, tag="qT")
            for oc in range(IC):
                ps = psum.tile([P, NT], f32, tag="ps_q", bufs=2)
                for i in range(IC):
                    nc.tensor.matmul(
                        ps[:],
                        wq_t[:, i, ts(oc, P)],
                        xT_t[:, i, :],
                        start=(i == 0),
                        stop=(i == IC - 1),
                    )
                if has_bq:
                    nc.vector.tensor_scalar_add(qT_t[:, oc, :], ps[:], bq_t[:, oc, :])
                else:
                    nc.vector.tensor_copy(qT_t[:, oc, :], ps[:])

            # attention: Otilde' rows 0-63 = unnormalized out, row 64 = denom.
            # oe_big slot layout: even heads 2c -> slot c, odd heads 2c+1 ->
            # slot IC+c, so den gather / stag / broadcast are 1-2 DMAs each.
            oe_big = work.tile([HD + 1, 2 * IC, NT], f32, tag="oext")
            den8_t = work.tile([2 * IC, NT], f32, tag="den8")
            for c in range(IC):  # head pair (2c, 2c+1); kT/qT chunk c
                es = {0: [], 1: []}
                for sc in range(SC):
                    ps_s = psum.tile([P, 2 * NT], f32, tag="ps_s", bufs=2)
                    for par in (0, 1):  # PE row groups 0-63 / 64-127
                        pslc = slice(par * HD, (par + 1) * HD)
                        nc.tensor.matmul(
                            ps_s[:, par * NT : (par + 1) * NT],
                            kT_t[pslc, c, ts(sc, P)],
                            qT_t[pslc, c, :],
                            start=True,
                            stop=True,
                        )
                    e = epool.tile([P, 2 * NT], mmdt, tag="e")
                    nc.scalar.activation(
                        e[:],
                        ps_s[:],
                        mybir.ActivationFunctionType.Exp,
                        bias=amask_t[:, sc, :],
                        scale=SCALE,
                    )
                    es[0].append(e[:, 0:NT])
                    es[1].append(e[:, NT : 2 * NT])

                for par in (0, 1):
                    h = 2 * c + par
                    ps_o = psum.tile([P, NT], f32, tag="ps_o", bufs=2)
                    for sc in range(SC):
                        nc.tensor.matmul(
                            ps_o[0 : HD + 1, :],
                            vext_t[:, sc, h, :],
                            es[par][sc],
                            start=(sc == 0),
                            stop=(sc == SC - 1),
                        )
                    nc.vector.tensor_copy(
                        oe_big[:, par * IC + c, :], ps_o[0 : HD + 1, :]
                    )

            # denominator path: gather (1 DMA) -> reciprocal (1 DVE op) ->
            # DRAM bounce (1 store + 2 broadcast DMAs) -> normalize muls.
            nc.gpsimd.dma_start(den8_t[:], oe_big[HD : HD + 1, :, :])
            nc.vector.reciprocal(den8_t[:], den8_t[:])
            nc.gpsimd.dma_start(rden_dram[t], den8_t[:])
            ot_t = work.tile([P, IC, NT], mmdt, tag="ot")
            stag_t = work.tile([P, IC, NT], f32, tag="stag")
            den_t = work.tile([P, IC, NT], f32, tag="den")
            for par in (0, 1):
                nc.gpsimd.dma_start(
                    den_t[par * HD : (par + 1) * HD, :, :],
                    rden_dram[t, par : par + 1].to_broadcast((HD, IC, NT)),
                )
            nc.gpsimd.dma_start(stag_t[HD:P, :, :], oe_big[0:HD, IC : 2 * IC, :])

            # previous tile's output projection slots in here: its inputs are
            # long ready, and it fills the PE while tile t's den DMAs fly.
            if prev is not None:
                emit_yproj(*prev)

            for c in range(IC):
                nc.vector.tensor_mul(
                    ot_t[0:HD, c, :], oe_big[0:HD, c, :], den_t[0:HD, c, :]
                )
                nc.vector.tensor_mul(
                    ot_t[HD:P, c, :], stag_t[HD:P, c, :], den_t[HD:P, c, :]
                )
            prev = (t, ot_t)

        emit_yproj(*prev)

    _split_multi_waits(nc)
    return nc


_NC_CACHE: dict = {}


def _get_nc(flags):
    if flags not in _NC_CACHE:
        _NC_CACHE[flags] = _build_nc(*flags)
    return _NC_CACHE[flags]


def _prep_in_maps(x, context, context_mask, wq, bq, wkv, bkv, wp, bp, mmdt_name=None):
    if mmdt_name is None:
        mmdt_name = MMDT_NAME
    np_mm = _np_mm(getattr(mybir.dt, mmdt_name))
    cvt = lambda a: np.ascontiguousarray(a).astype(np_mm, copy=False)
    wqT = cvt(wq.T)
    wkT = cvt(wkv[:D].T)
    wvT = cvt(wkv[D:].T)
    wpT = cvt(wp.T)
    bq_c = np.ascontiguousarray(bq.reshape(D, 1), dtype=np.float32)
    bk_c = np.ascontiguousarray(bkv[:D].reshape(D, 1), dtype=np.float32)
    bv_r = cvt(bkv[D:].reshape(1, D))
    bp_r = cvt(bp.reshape(1, D))
    flags = (
        mmdt_name,
        bool(np.any(bq != 0)),
        bool(np.any(bkv[:D] != 0)),
        bool(np.any(bkv[D:] != 0)),
        bool(np.any(bp != 0)),
    )
    in_maps = []
    for b in range(B):
        amask = np.where(context_mask[b], np.float32(MASK_NEG), np.float32(0.0))
        in_maps.append(
            {
                "xT": cvt(x[b].T),
                "ctxT": cvt(context[b].T),
                "wqT": wqT,
                "wkT": wkT,
                "wvT": wvT,
                "wpT": wpT,
                "bq": bq_c,
                "bk": bk_c,
                "bv": bv_r,
                "bp": bp_r,
                "amask": amask.astype(np.float32).reshape(S, 1),
            }
        )
    return in_maps, flags


def kernel(x, context, context_mask, wq, bq, wkv, bkv, wp, bp):
    from concourse.bass_utils import run_bass_kernel_spmd

    in_maps, flags = _prep_in_maps(
        x, context, context_mask, wq, bq, wkv, bkv, wp, bp
    )
    nc = _get_nc(flags)
    res = run_bass_kernel_spmd(nc, in_maps, list(range(B)))
    return np.stack([np.asarray(res.results[b]["y"]) for b in range(B)], axis=0)



# revision 23
# speedup vs baseline: 1.6045x; 1.1309x over previous
"""Trainium2 Bass kernel for nn_CrossAttention (B=8, N=4096, S=512, D=512, H=8).

Sharding: data-parallel over batch — each of the 8 NeuronCores computes the
full cross-attention for one batch element. No collectives needed.

Key optimizations over a straightforward port:
  - bf16 matmul inputs (fp32 PSUM accumulation): 1 cycle/column on the PE
    vs 2 for fp32r.
  - context compaction: masked context tokens (about half; mask True =
    padding) are dropped on the host, so scores / exp / attn@V only run
    over ceil(SEFF/128) chunks instead of 4. Padding inside the last chunk
    is killed by the exp bias (-30000 -> e==0).
  - the exp for both heads of a pair goes through one [128, 2*NT] PSUM
    tile -> one ACT instruction (amortizes ACT fixed overhead).
  - softmax denominators: the attn@V matmul computes them via an appended
    ones-column (M=65); all 8 rows are gathered by 1 DMA into [8, NT],
    inverted by ONE DVE reciprocal (its cost is free-dim-bound), and
    broadcast back across partitions via a DRAM bounce (2 DMAs).
  - software pipelining: tile t's output projection is emitted after tile
    t+1's attention matmuls, so the in-order PE queue never waits on the
    denominator round-trip.
  - normalize multiplies are 4x [128, NT] (both heads of a pair merged;
    odd/even halves staged into one tile by 2 DMAs).
  - engine balancing: y-tile PSUM evacuation on ACT (copy lives in the
    same act table as exp), normalize/qT work on DVE, all small DMAs
    issued from the otherwise idle Pool queue.

Per-core dataflow (all activations feature-major so no transposes):
  xT [D, N] (host-transposed), ctxT [D, SEFF] (host-compacted):
  qT = wqT.T@xT, kT = wkT.T@ctxT (feature-major), v token-major with ones
  column -> scoresT [s, n] per head -> e = exp(SCALE*scores + mask_bias)
  -> Otilde'[{d,den}, n] = vext.T @ e -> normalize -> y = OT.T @ wpT.
"""

import os

import numpy as np

try:
    import concourse.bass as bass
except ImportError:
    import sys

    sys.path.insert(0, "/opt/trn_rl_repo")
    import concourse.bass as bass

from contextlib import ExitStack

import concourse.mybir as mybir
import concourse.tile as tile
from concourse.bass import ts

B, N, S, D, H = 8, 4096, 512, 512, 8
HD = D // H  # 64
SCALE = HD**-0.5
P = 128
IC = D // P  # 4 chunks of the contraction/feature dims
NT = 512  # queries per outer tile
NTILES = N // NT  # 8
NSUB = NT // P  # 4
MASK_NEG = -30000.0

f32 = mybir.dt.float32

MMDT_NAME = os.environ.get("KMMDT", "bfloat16")


def _np_mm(mmdt):
    return np.dtype(mybir.dt.np(mmdt))


def _split_multi_waits(nc: bass.Bass) -> None:
    """This walrus toolchain accepts at most ONE sync-wait per instruction
    ("Too many sync wait commands" in setupSyncWait, seen for MM/LW, NoOp,
    and DMA structs alike). Hoist all but the last wait of any instruction
    onto a chain of same-engine InstNoOps spliced immediately before it —
    same program position, so synchronization semantics are unchanged."""
    eng_map = {
        mybir.EngineType.PE: lambda: nc.tensor,
        mybir.EngineType.Activation: lambda: nc.scalar,
        mybir.EngineType.DVE: lambda: nc.vector,
        mybir.EngineType.Pool: lambda: nc.gpsimd,
        mybir.EngineType.SP: lambda: nc.sync,
    }
    for fn in nc.m.functions:
        blocks = fn.blocks
        for bb in blocks:
            insts = list(bb.instructions)
            out = []
            changed = False
            for inst in insts:
                si = inst.sync_info
                if (
                    si is not None
                    and len(si.on_wait) > 1
                    and inst.engine in eng_map
                ):
                    waits = list(si.on_wait)
                    for w in waits[:-1]:  # one nop per excess wait
                        nop = eng_map[inst.engine]().nop(nofuse=True).ins
                        # the nop was appended to whatever block is current;
                        # strip it from there before splicing it in place
                        for bb2 in blocks:
                            lst = list(bb2.instructions)
                            if any(x.name == nop.name for x in lst):
                                bb2.instructions = [
                                    x for x in lst if x.name != nop.name
                                ]
                                if bb2 is bb:
                                    insts = [
                                        x for x in insts if x.name != nop.name
                                    ]
                        nop.sync_info = mybir.SyncInfo(
                            on_wait=[w], on_update=[]
                        )
                        out.append(nop)
                    inst.sync_info = mybir.SyncInfo(
                        on_wait=waits[-1:], on_update=list(si.on_update)
                    )
                    changed = True
                out.append(inst)
            if changed:
                bb.instructions = out


def _build_nc(mmdt_name: str, seff: int, has_bq, has_bk, has_bv, has_bp) -> bass.Bass:
    mmdt = getattr(mybir.dt, mmdt_name)
    nc = bass.Bass()

    SEFF = seff
    NCH = (SEFF + P - 1) // P  # context chunks
    CS = [min(P, SEFF - i * P) for i in range(NCH)]  # chunk sizes
    SEFF_PAD = NCH * P

    xT = nc.dram_tensor("xT", [D, N], mmdt, kind="ExternalInput")
    ctxT = nc.dram_tensor("ctxT", [D, SEFF], mmdt, kind="ExternalInput")
    wqT = nc.dram_tensor("wqT", [D, D], mmdt, kind="ExternalInput")
    wkT = nc.dram_tensor("wkT", [D, D], mmdt, kind="ExternalInput")
    wvT = nc.dram_tensor("wvT", [D, D], mmdt, kind="ExternalInput")
    wpT = nc.dram_tensor("wpT", [D, D], mmdt, kind="ExternalInput")
    bq = nc.dram_tensor("bq", [D, 1], f32, kind="ExternalInput")
    bk = nc.dram_tensor("bk", [D, 1], f32, kind="ExternalInput")
    bv = nc.dram_tensor("bv", [1, D], mmdt, kind="ExternalInput")
    bp = nc.dram_tensor("bp", [1, D], mmdt, kind="ExternalInput")
    amask = nc.dram_tensor("amask", [SEFF_PAD, 1], f32, kind="ExternalInput")
    y = nc.dram_tensor("y", [N, D], f32, kind="ExternalOutput")

    # reciprocal denominators, head-permuted: [tile, par, c, n] where head
    # h = 2c+par lives at [par, c] -- so one broadcast DMA serves each par.
    rden_dram = nc.dram_tensor("rden_scratch", [NTILES, 2, IC, NT], f32)

    ch = lambda dram: dram.rearrange("(c p) o -> p c o", p=P)  # [P, IC, D]

    with tile.TileContext(nc) as tc, ExitStack() as ctx:
        const = ctx.enter_context(tc.tile_pool(name="const", bufs=1))
        work = ctx.enter_context(tc.tile_pool(name="work", bufs=2))
        epool = ctx.enter_context(tc.tile_pool(name="epool", bufs=12))
        ypool = ctx.enter_context(tc.tile_pool(name="ypool", bufs=4))
        psum = ctx.enter_context(tc.tile_pool(name="psum", bufs=1, space="PSUM"))

        # ---- persistent tiles -------------------------------------------
        wq_t = const.tile([P, IC, D], mmdt)
        wk_t = const.tile([P, IC, D], mmdt)
        wv_t = const.tile([P, IC, D], mmdt)
        wp_t = const.tile([P, IC, D], mmdt)
        ctx_t = const.tile([P, IC, SEFF], mmdt)
        amask_t = const.tile([P, NCH, 1], f32)
        nc.sync.dma_start(wq_t[:], ch(wqT))
        nc.sync.dma_start(wk_t[:], ch(wkT))
        nc.sync.dma_start(wv_t[:], ch(wvT))
        nc.sync.dma_start(wp_t[:], ch(wpT))
        nc.sync.dma_start(ctx_t[:], ch(ctxT))
        nc.sync.dma_start(amask_t[:], amask.rearrange("(c p) o -> p c o", p=P))

        if has_bq:
            bq_t = const.tile([P, IC, 1], f32)
            nc.sync.dma_start(bq_t[:], bq.rearrange("(c p) o -> p c o", p=P))
        if has_bk:
            bk_t = const.tile([P, IC, 1], f32)
            nc.sync.dma_start(bk_t[:], bk.rearrange("(c p) o -> p c o", p=P))
        if has_bv or has_bp:
            ones1_t = const.tile([1, P], mmdt)
            nc.vector.memset(ones1_t[:], 1.0)
        if has_bv:
            bv_t = const.tile([1, D], mmdt)
            nc.sync.dma_start(bv_t[:], bv[:])
        if has_bp:
            bp_t = const.tile([1, D], mmdt)
            nc.sync.dma_start(bp_t[:], bp[:])

        kT_t = const.tile([P, IC, SEFF], mmdt)  # feature-major keys
        vext_t = const.tile([P, NCH, H, HD + 1], mmdt)  # token-major v + ones

        # ---- kv projections (once per core) -----------------------------
        for sc in range(NCH):
            for h in range(H):
                nc.vector.memset(vext_t[0 : CS[sc], sc, h, HD : HD + 1], 1.0)

        for kc in range(IC):  # dk chunks -> kT
            ps = psum.tile([P, NT], f32, tag="ps_q", bufs=2)
            for i in range(IC):
                nc.tensor.matmul(
                    ps[:, 0:SEFF],
                    wk_t[:, i, ts(kc, P)],
                    ctx_t[:, i, :],
                    start=(i == 0),
                    stop=(i == IC - 1),
                )
            if has_bk:
                nc.vector.tensor_scalar_add(kT_t[:, kc, :], ps[:, 0:SEFF], bk_t[:, kc, :])
            else:
                nc.vector.tensor_copy(kT_t[:, kc, :], ps[:, 0:SEFF])

        for sc in range(NCH):  # s chunks -> v (token-major)
            csz = CS[sc]
            ps = psum.tile([P, D], f32, tag="ps_o", bufs=2)
            for i in range(IC):
                nc.tensor.matmul(
                    ps[0:csz, :],
                    ctx_t[:, i, sc * P : sc * P + csz],
                    wv_t[:, i, :],
                    start=(i == 0),
                    stop=(i == IC - 1 and not has_bv),
                )
            if has_bv:
                nc.tensor.matmul(
                    ps[0:csz, :], ones1_t[:, 0:csz], bv_t[:], start=False, stop=True
                )
            for h in range(H):
                nc.vector.tensor_copy(
                    vext_t[0:csz, sc, h, 0:HD], ps[0:csz, h * HD : (h + 1) * HD]
                )

        # ---- main loop over query tiles ---------------------------------
        # Software-pipelined: tile t's output projection is emitted AFTER
        # tile t+1's attention matmuls so the PE never sits behind tile t's
        # denominator round-trip (engine queues are in-order; a y-proj
        # waiting on the normalize would head-of-line-block the next tile).
        def emit_yproj(t, ot_t):
            for ns in range(NSUB):
                ps_y = psum.tile([P, D], f32, tag="ps_o", bufs=2)
                for c in range(IC):
                    nc.tensor.matmul(
                        ps_y[:],
                        ot_t[:, c, ts(ns, P)],
                        wp_t[:, c, :],
                        start=(c == 0),
                        stop=(c == IC - 1 and not has_bp),
                    )
                if has_bp:
                    nc.tensor.matmul(ps_y[:], ones1_t[:], bp_t[:], start=False, stop=True)
                y_t = ypool.tile([P, D], f32, tag="y")
                nc.scalar.activation(
                    y_t[:], ps_y[:], mybir.ActivationFunctionType.Copy
                )
                nc.sync.dma_start(y[t * NT + ns * P : t * NT + (ns + 1) * P, :], y_t[:])

        prev = None  # (t, ot_t) awaiting output projection
        for t in range(NTILES):
            xT_t = work.tile([P, IC, NT], mmdt, tag="xT")
            nc.sync.dma_start(
                xT_t[:], xT[:, ts(t, NT)].rearrange("(c p) n -> p c n", p=P)
            )

            # qT for this tile (feature-major)
            qT_t = work.tile([P, IC, NT], mmdt, tag="qT")
            for oc in range(IC):
                ps = psum.tile([P, NT], f32, tag="ps_q", bufs=2)
                for i in range(IC):
                    nc.tensor.matmul(
                        ps[:],
                        wq_t[:, i, ts(oc, P)],
                        xT_t[:, i, :],
                        start=(i == 0),
                        stop=(i == IC - 1),
                    )
                if has_bq:
                    nc.vector.tensor_scalar_add(qT_t[:, oc, :], ps[:], bq_t[:, oc, :])
                else:
                    nc.vector.tensor_copy(qT_t[:, oc, :], ps[:])

            # attention: Otilde' rows 0-63 = unnormalized out, row 64 = denom.
            # oe_big slot layout: even heads 2c -> slot c, odd heads 2c+1 ->
            # slot IC+c, so den gather / stag / broadcast are 1-2 DMAs each.
            oe_big = work.tile([HD + 1, 2 * IC, NT], f32, tag="oext")
            den8_t = work.tile([2 * IC, NT], f32, tag="den8")
            for c in range(IC):  # head pair (2c, 2c+1); kT/qT chunk c
                es = {0: [], 1: []}
                for sc in range(NCH):
                    csz = CS[sc]
                    ps_s = psum.tile([P, 2 * NT], f32, tag="ps_s", bufs=2)
                    for par in (0, 1):  # PE row groups 0-63 / 64-127
                        pslc = slice(par * HD, (par + 1) * HD)
                        nc.tensor.matmul(
                            ps_s[0:csz, par * NT : (par + 1) * NT],
                            kT_t[pslc, c, sc * P : sc * P + csz],
                            qT_t[pslc, c, :],
                            start=True,
                            stop=True,
                        )
                    e = epool.tile([P, 2 * NT], mmdt, tag="e")
                    nc.scalar.activation(
                        e[0:csz, :],
                        ps_s[0:csz, :],
                        mybir.ActivationFunctionType.Exp,
                        bias=amask_t[0:csz, sc, :],
                        scale=SCALE,
                    )
                    es[0].append(e[0:csz, 0:NT])
                    es[1].append(e[0:csz, NT : 2 * NT])

                for par in (0, 1):
                    h = 2 * c + par
                    ps_o = psum.tile([P, NT], f32, tag="ps_o", bufs=2)
                    for sc in range(NCH):
                        nc.tensor.matmul(
                            ps_o[0 : HD + 1, :],
                            vext_t[0 : CS[sc], sc, h, :],
                            es[par][sc],
                            start=(sc == 0),
                            stop=(sc == NCH - 1),
                        )
                    nc.vector.tensor_copy(
                        oe_big[:, par * IC + c, :], ps_o[0 : HD + 1, :]
                    )

            # denominator path: gather (1 DMA) -> reciprocal (1 DVE op) ->
            # DRAM bounce (1 store + 2 broadcast DMAs) -> normalize muls.
            nc.gpsimd.dma_start(den8_t[:], oe_big[HD : HD + 1, :, :])
            nc.vector.reciprocal(den8_t[:], den8_t[:])
            nc.gpsimd.dma_start(rden_dram[t], den8_t[:])
            ot_t = work.tile([P, IC, NT], mmdt, tag="ot")
            numer_t = work.tile([P, IC, NT], f32, tag="numer")
            den_t = work.tile([P, IC, NT], f32, tag="den")
            for par in (0, 1):
                nc.gpsimd.dma_start(
                    den_t[par * HD : (par + 1) * HD, :, :],
                    rden_dram[t, par : par + 1].to_broadcast((HD, IC, NT)),
                )
                nc.gpsimd.dma_start(
                    numer_t[par * HD : (par + 1) * HD, :, :],
                    oe_big[0:HD, par * IC : (par + 1) * IC, :],
                )

            # previous tile's output projection slots in here: its inputs are
            # long ready, and it fills the PE while tile t's den DMAs fly.
            if prev is not None:
                emit_yproj(*prev)

            for c in range(IC):
                nc.vector.tensor_mul(
                    ot_t[:, c, :], numer_t[:, c, :], den_t[:, c, :]
                )
            prev = (t, ot_t)

        emit_yproj(*prev)

    _split_multi_waits(nc)
    return nc


_NC_CACHE: dict = {}


def _get_nc(flags):
    if flags not in _NC_CACHE:
        _NC_CACHE[flags] = _build_nc(*flags)
    return _NC_CACHE[flags]


def _prep_in_maps(x, context, context_mask, wq, bq, wkv, bkv, wp, bp, mmdt_name=None):
    if mmdt_name is None:
        mmdt_name = MMDT_NAME
    np_mm = _np_mm(getattr(mybir.dt, mmdt_name))
    cvt = lambda a: np.ascontiguousarray(a).astype(np_mm, copy=False)

    # --- context compaction: drop masked (True = padding) tokens ---------
    keep = [np.where(~context_mask[b])[0] for b in range(B)]
    cnts = [len(k) for k in keep]
    seff = max(16, -(-max(cnts) // 16) * 16)  # round up to mult of 16
    seff = min(seff, S)
    seff_pad = -(-seff // 128) * 128

    wqT = cvt(wq.T)
    wkT = cvt(wkv[:D].T)
    wvT = cvt(wkv[D:].T)
    wpT = cvt(wp.T)
    bq_c = np.ascontiguousarray(bq.reshape(D, 1), dtype=np.float32)
    bk_c = np.ascontiguousarray(bkv[:D].reshape(D, 1), dtype=np.float32)
    bv_r = cvt(bkv[D:].reshape(1, D))
    bp_r = cvt(bp.reshape(1, D))
    flags = (
        mmdt_name,
        int(seff),
        bool(np.any(bq != 0)),
        bool(np.any(bkv[:D] != 0)),
        bool(np.any(bkv[D:] != 0)),
        bool(np.any(bp != 0)),
    )
    in_maps = []
    for b in range(B):
        cnt = cnts[b]
        ctxc = np.zeros((D, seff), dtype=np_mm)
        ctxc[:, :cnt] = cvt(context[b][keep[b]].T)
        am = np.full((seff_pad, 1), np.float32(MASK_NEG), dtype=np.float32)
        am[:cnt] = 0.0
        in_maps.append(
            {
                "xT": cvt(x[b].T),
                "ctxT": ctxc,
                "wqT": wqT,
                "wkT": wkT,
                "wvT": wvT,
                "wpT": wpT,
                "bq": bq_c,
                "bk": bk_c,
                "bv": bv_r,
                "bp": bp_r,
                "amask": am,
            }
        )
    return in_maps, flags


def kernel(x, context, context_mask, wq, bq, wkv, bkv, wp, bp):
    from concourse.bass_utils import run_bass_kernel_spmd

    in_maps, flags = _prep_in_maps(
        x, context, context_mask, wq, bq, wkv, bkv, wp, bp
    )
    nc = _get_nc(flags)
    res = run_bass_kernel_spmd(nc, in_maps, list(range(B)))
    return np.stack([np.asarray(res.results[b]["y"]) for b in range(B)], axis=0)


# revision 27
# speedup vs baseline: 1.6068x; 1.0015x over previous
"""Trainium2 Bass kernel for nn_CrossAttention (B=8, N=4096, S=512, D=512, H=8).

Sharding: data-parallel over batch — each of the 8 NeuronCores computes the
full cross-attention for one batch element. No collectives needed.

Key optimizations over a straightforward port:
  - bf16 matmul inputs (fp32 PSUM accumulation): 1 cycle/column on the PE
    vs 2 for fp32r.
  - context compaction: masked context tokens (about half; mask True =
    padding) are dropped on the host, so scores / exp / attn@V only run
    over ceil(SEFF/128) chunks instead of 4. Padding inside the last chunk
    is killed by the exp bias (-30000 -> e==0).
  - the exp for both heads of a pair goes through one [128, 2*NT] PSUM
    tile -> one ACT instruction (amortizes ACT fixed overhead).
  - softmax denominators: the attn@V matmul computes them via an appended
    ones-column (M=65); all 8 rows are gathered by 1 DMA into [8, NT],
    inverted by ONE DVE reciprocal (its cost is free-dim-bound), and
    broadcast back across partitions via a DRAM bounce (2 DMAs).
  - software pipelining: tile t's output projection is emitted after tile
    t+1's attention matmuls, so the in-order PE queue never waits on the
    denominator round-trip.
  - normalize multiplies are 4x [128, NT] (both heads of a pair merged;
    odd/even halves staged into one tile by 2 DMAs).
  - engine balancing: y-tile PSUM evacuation on ACT (copy lives in the
    same act table as exp), normalize/qT work on DVE, all small DMAs
    issued from the otherwise idle Pool queue.

Per-core dataflow (all activations feature-major so no transposes):
  xT [D, N] (host-transposed), ctxT [D, SEFF] (host-compacted):
  qT = wqT.T@xT, kT = wkT.T@ctxT (feature-major), v token-major with ones
  column -> scoresT [s, n] per head -> e = exp(SCALE*scores + mask_bias)
  -> Otilde'[{d,den}, n] = vext.T @ e -> normalize -> y = OT.T @ wpT.
"""

import os

import numpy as np

try:
    import concourse.bass as bass
except ImportError:
    import sys

    sys.path.insert(0, "/opt/trn_rl_repo")
    import concourse.bass as bass

from contextlib import ExitStack

import concourse.mybir as mybir
import concourse.tile as tile
from concourse.bass import ts

B, N, S, D, H = 8, 4096, 512, 512, 8
HD = D // H  # 64
SCALE = HD**-0.5
P = 128
IC = D // P  # 4 chunks of the contraction/feature dims
NT = 512  # queries per outer tile
NTILES = N // NT  # 8
NSUB = NT // P  # 4
MASK_NEG = -30000.0

f32 = mybir.dt.float32

MMDT_NAME = os.environ.get("KMMDT", "bfloat16")


def _np_mm(mmdt):
    return np.dtype(mybir.dt.np(mmdt))


def _split_multi_waits(nc: bass.Bass) -> None:
    """This walrus toolchain accepts at most ONE sync-wait per instruction
    ("Too many sync wait commands" in setupSyncWait, seen for MM/LW, NoOp,
    and DMA structs alike). Hoist all but the last wait of any instruction
    onto a chain of same-engine InstNoOps spliced immediately before it —
    same program position, so synchronization semantics are unchanged."""
    eng_map = {
        mybir.EngineType.PE: lambda: nc.tensor,
        mybir.EngineType.Activation: lambda: nc.scalar,
        mybir.EngineType.DVE: lambda: nc.vector,
        mybir.EngineType.Pool: lambda: nc.gpsimd,
        mybir.EngineType.SP: lambda: nc.sync,
    }
    for fn in nc.m.functions:
        blocks = fn.blocks
        for bb in blocks:
            insts = list(bb.instructions)
            out = []
            changed = False
            for inst in insts:
                si = inst.sync_info
                if (
                    si is not None
                    and len(si.on_wait) > 1
                    and inst.engine in eng_map
                ):
                    waits = list(si.on_wait)
                    for w in waits[:-1]:  # one nop per excess wait
                        nop = eng_map[inst.engine]().nop(nofuse=True).ins
                        # the nop was appended to whatever block is current;
                        # strip it from there before splicing it in place
                        for bb2 in blocks:
                            lst = list(bb2.instructions)
                            if any(x.name == nop.name for x in lst):
                                bb2.instructions = [
                                    x for x in lst if x.name != nop.name
                                ]
                                if bb2 is bb:
                                    insts = [
                                        x for x in insts if x.name != nop.name
                                    ]
                        nop.sync_info = mybir.SyncInfo(
                            on_wait=[w], on_update=[]
                        )
                        out.append(nop)
                    inst.sync_info = mybir.SyncInfo(
                        on_wait=waits[-1:], on_update=list(si.on_update)
                    )
                    changed = True
                out.append(inst)
            if changed:
                bb.instructions = out


def _build_nc(mmdt_name: str, seff: int, has_bq, has_bk, has_bv, has_bp) -> bass.Bass:
    mmdt = getattr(mybir.dt, mmdt_name)
    nc = bass.Bass()

    SEFF = seff
    NCH = (SEFF + P - 1) // P  # context chunks
    CS = [min(P, SEFF - i * P) for i in range(NCH)]  # chunk sizes
    SEFF_PAD = NCH * P

    xT = nc.dram_tensor("xT", [D, N], mmdt, kind="ExternalInput")
    ctxT = nc.dram_tensor("ctxT", [D, SEFF], mmdt, kind="ExternalInput")
    wqT = nc.dram_tensor("wqT", [D, D], mmdt, kind="ExternalInput")
    wkT = nc.dram_tensor("wkT", [D, D], mmdt, kind="ExternalInput")
    wvT = nc.dram_tensor("wvT", [D, D], mmdt, kind="ExternalInput")
    wpT = nc.dram_tensor("wpT", [D, D], mmdt, kind="ExternalInput")
    bq = nc.dram_tensor("bq", [D, 1], f32, kind="ExternalInput")
    bk = nc.dram_tensor("bk", [D, 1], f32, kind="ExternalInput")
    bv = nc.dram_tensor("bv", [1, D], mmdt, kind="ExternalInput")
    bp = nc.dram_tensor("bp", [1, D], mmdt, kind="ExternalInput")
    amask = nc.dram_tensor("amask", [SEFF_PAD, 1], f32, kind="ExternalInput")
    y = nc.dram_tensor("y", [N, D], f32, kind="ExternalOutput")

    # reciprocal denominators, head-permuted: [tile, par, c, n] where head
    # h = 2c+par lives at [par, c] -- so one broadcast DMA serves each par.
    rden_dram = nc.dram_tensor("rden_scratch", [NTILES, 2, IC, NT], f32)

    ch = lambda dram: dram.rearrange("(c p) o -> p c o", p=P)  # [P, IC, D]

    with tile.TileContext(nc) as tc, ExitStack() as ctx:
        const = ctx.enter_context(tc.tile_pool(name="const", bufs=1))
        work = ctx.enter_context(tc.tile_pool(name="work", bufs=2))
        epool = ctx.enter_context(tc.tile_pool(name="epool", bufs=12))
        ypool = ctx.enter_context(tc.tile_pool(name="ypool", bufs=4))
        psum = ctx.enter_context(tc.tile_pool(name="psum", bufs=1, space="PSUM"))

        # ---- persistent tiles -------------------------------------------
        wq_t = const.tile([P, IC, D], mmdt)
        wk_t = const.tile([P, IC, D], mmdt)
        wv_t = const.tile([P, IC, D], mmdt)
        wp_t = const.tile([P, IC, D], mmdt)
        ctx_t = const.tile([P, IC, SEFF], mmdt)
        amask_t = const.tile([P, NCH, 1], f32)
        nc.sync.dma_start(wq_t[:], ch(wqT))
        nc.sync.dma_start(wk_t[:], ch(wkT))
        nc.sync.dma_start(wv_t[:], ch(wvT))
        nc.sync.dma_start(wp_t[:], ch(wpT))
        nc.sync.dma_start(ctx_t[:], ch(ctxT))
        nc.sync.dma_start(amask_t[:], amask.rearrange("(c p) o -> p c o", p=P))

        if has_bq:
            bq_t = const.tile([P, IC, 1], f32)
            nc.sync.dma_start(bq_t[:], bq.rearrange("(c p) o -> p c o", p=P))
        if has_bk:
            bk_t = const.tile([P, IC, 1], f32)
            nc.sync.dma_start(bk_t[:], bk.rearrange("(c p) o -> p c o", p=P))
        if has_bv or has_bp:
            ones1_t = const.tile([1, P], mmdt)
            nc.vector.memset(ones1_t[:], 1.0)
        if has_bv:
            bv_t = const.tile([1, D], mmdt)
            nc.sync.dma_start(bv_t[:], bv[:])
        if has_bp:
            bp_t = const.tile([1, D], mmdt)
            nc.sync.dma_start(bp_t[:], bp[:])

        kT_t = const.tile([P, IC, SEFF], mmdt)  # feature-major keys
        vext_t = const.tile([P, NCH, H, HD + 1], mmdt)  # token-major v + ones
        # packed-tail attnV needs lhsT at the same base partition as the
        # packed e slices (32c), so the tail v rows are replicated 4x
        vext_tail4 = const.tile([P, H, HD + 1], mmdt)

        # ---- kv projections (once per core) -----------------------------
        packed_tail = NCH >= 2 and CS[-1] <= 32 and IC * 32 <= P
        FCH = NCH - 1 if packed_tail else NCH  # chunks handled per-pair
        for sc in range(FCH):
            for h in range(H):
                nc.vector.memset(vext_t[0 : CS[sc], sc, h, HD : HD + 1], 1.0)
        if packed_tail:
            for h in range(H):
                for c in range(IC):
                    nc.vector.memset(
                        vext_tail4[c * 32 : c * 32 + CS[-1], h, HD : HD + 1], 1.0
                    )

        for kc in range(IC):  # dk chunks -> kT
            ps = psum.tile([P, NT], f32, tag="ps_q", bufs=2)
            for i in range(IC):
                nc.tensor.matmul(
                    ps[:, 0:SEFF],
                    wk_t[:, i, ts(kc, P)],
                    ctx_t[:, i, :],
                    start=(i == 0),
                    stop=(i == IC - 1),
                )
            if has_bk:
                nc.vector.tensor_scalar_add(kT_t[:, kc, :], ps[:, 0:SEFF], bk_t[:, kc, :])
            else:
                nc.vector.tensor_copy(kT_t[:, kc, :], ps[:, 0:SEFF])

        for sc in range(NCH):  # s chunks -> v (token-major)
            csz = CS[sc]
            ps = psum.tile([P, D], f32, tag="ps_o", bufs=2)
            for i in range(IC):
                nc.tensor.matmul(
                    ps[0:csz, :],
                    ctx_t[:, i, sc * P : sc * P + csz],
                    wv_t[:, i, :],
                    start=(i == 0),
                    stop=(i == IC - 1 and not has_bv),
                )
            if has_bv:
                nc.tensor.matmul(
                    ps[0:csz, :], ones1_t[:, 0:csz], bv_t[:], start=False, stop=True
                )
            for h in range(H):
                if packed_tail and sc == NCH - 1:
                    for c in range(IC):
                        nc.vector.tensor_copy(
                            vext_tail4[c * 32 : c * 32 + csz, h, 0:HD],
                            ps[0:csz, h * HD : (h + 1) * HD],
                        )
                else:
                    nc.vector.tensor_copy(
                        vext_t[0:csz, sc, h, 0:HD], ps[0:csz, h * HD : (h + 1) * HD]
                    )

        # ---- main loop over query tiles ---------------------------------
        # Software-pipelined two tiles deep: tile t's normalize multiplies
        # are emitted after tile t+1's q-projection (so the DVE never waits
        # on t's denominator broadcast), and tile t's output projection is
        # emitted after tile t+1's attention matmuls (so the in-order PE
        # queue never waits on the normalize).
        # Tail packing: when the last context chunk is small (csz*IC <= 128)
        # the 4 head-pairs' tail scores go into ONE psum tile at partition
        # offsets c*csz, and a single EXP (with a x4-replicated mask bias)
        # covers them all.
        # tail slots sit at 32-partition boundaries (matmul output base
        # partition must be a multiple of 32)
        if packed_tail:
            tcsz = CS[-1]
            amask_tail = const.tile([IC * 32, 1], f32)
            nc.vector.memset(amask_tail[:], MASK_NEG)
            for c in range(IC):
                nc.gpsimd.dma_start(
                    amask_tail[c * 32 : c * 32 + tcsz, :],
                    amask[FCH * P : FCH * P + tcsz, :],
                )

        def emit_muls(t, ot_t, numer_t, den_t):
            for c in range(IC):
                nc.vector.tensor_mul(
                    ot_t[:, c, :], numer_t[:, c, :], den_t[:, c, :]
                )

        def emit_yproj(t, ot_t):
            for ns in range(NSUB):
                ps_y = psum.tile([P, D], f32, tag="ps_o", bufs=2)
                for c in range(IC):
                    nc.tensor.matmul(
                        ps_y[:],
                        ot_t[:, c, ts(ns, P)],
                        wp_t[:, c, :],
                        start=(c == 0),
                        stop=(c == IC - 1 and not has_bp),
                    )
                if has_bp:
                    nc.tensor.matmul(ps_y[:], ones1_t[:], bp_t[:], start=False, stop=True)
                y_t = ypool.tile([P, D], f32, tag="y")
                nc.scalar.activation(
                    y_t[:], ps_y[:], mybir.ActivationFunctionType.Copy
                )
                nc.sync.dma_start(y[t * NT + ns * P : t * NT + (ns + 1) * P, :], y_t[:])

        pend = None  # (t, ot_t, numer_t, den_t) awaiting normalize + y-proj
        for t in range(NTILES):
            xT_t = work.tile([P, IC, NT], mmdt, tag="xT")
            nc.sync.dma_start(
                xT_t[:], xT[:, ts(t, NT)].rearrange("(c p) n -> p c n", p=P)
            )

            # qT for this tile (feature-major)
            qT_t = work.tile([P, IC, NT], mmdt, tag="qT")
            for oc in range(IC):
                ps = psum.tile([P, NT], f32, tag="ps_q", bufs=2)
                for i in range(IC):
                    nc.tensor.matmul(
                        ps[:],
                        wq_t[:, i, ts(oc, P)],
                        xT_t[:, i, :],
                        start=(i == 0),
                        stop=(i == IC - 1),
                    )
                if has_bq:
                    nc.vector.tensor_scalar_add(qT_t[:, oc, :], ps[:], bq_t[:, oc, :])
                else:
                    nc.vector.tensor_copy(qT_t[:, oc, :], ps[:])

            # previous tile's normalize multiplies: den broadcast has had a
            # full q-projection to land, so the DVE takes these stall-free.
            if pend is not None:
                emit_muls(*pend)

            # packed tail scores + one EXP for all 4 head-pairs
            e_tail = None
            if packed_tail:
                ps_st = psum.tile([P, 2 * NT], f32, tag="ps_s", bufs=2)
                for c in range(IC):
                    for par in (0, 1):
                        pslc = slice(par * HD, (par + 1) * HD)
                        nc.tensor.matmul(
                            ps_st[c * 32 : c * 32 + tcsz,
                                  par * NT : (par + 1) * NT],
                            kT_t[pslc, c, FCH * P : FCH * P + tcsz],
                            qT_t[pslc, c, :],
                            start=True,
                            stop=True,
                            tile_position=(par * HD, c * 32),
                        )
                e_tail = epool.tile([P, 2 * NT], mmdt, tag="e")
                nc.scalar.activation(
                    e_tail[0 : IC * 32, :],
                    ps_st[0 : IC * 32, :],
                    mybir.ActivationFunctionType.Exp,
                    bias=amask_tail[:],
                    scale=SCALE,
                )

            # attention: Otilde' rows 0-63 = unnormalized out, row 64 = denom.
            # oe_big slot layout: even heads 2c -> slot c, odd heads 2c+1 ->
            # slot IC+c, so den gather / stag / broadcast are 1-2 DMAs each.
            oe_big = work.tile([HD + 1, 2 * IC, NT], f32, tag="oext")
            den8_t = work.tile([2 * IC, NT], f32, tag="den8")
            for c in range(IC):  # head pair (2c, 2c+1); kT/qT chunk c
                es = {0: [], 1: []}
                for sc in range(FCH):
                    csz = CS[sc]
                    ps_s = psum.tile([P, 2 * NT], f32, tag="ps_s", bufs=2)
                    for par in (0, 1):  # PE row groups 0-63 / 64-127
                        pslc = slice(par * HD, (par + 1) * HD)
                        nc.tensor.matmul(
                            ps_s[0:csz, par * NT : (par + 1) * NT],
                            kT_t[pslc, c, sc * P : sc * P + csz],
                            qT_t[pslc, c, :],
                            start=True,
                            stop=True,
                        )
                    e = epool.tile([P, 2 * NT], mmdt, tag="e")
                    nc.scalar.activation(
                        e[0:csz, :],
                        ps_s[0:csz, :],
                        mybir.ActivationFunctionType.Exp,
                        bias=amask_t[0:csz, sc, :],
                        scale=SCALE,
                    )
                    es[0].append(e[0:csz, 0:NT])
                    es[1].append(e[0:csz, NT : 2 * NT])
                if packed_tail:
                    es[0].append(e_tail[c * 32 : c * 32 + tcsz, 0:NT])
                    es[1].append(e_tail[c * 32 : c * 32 + tcsz, NT : 2 * NT])

                for par in (0, 1):
                    h = 2 * c + par
                    ps_o = psum.tile([P, NT], f32, tag="ps_o", bufs=2)
                    for sc in range(NCH):
                        if packed_tail and sc == NCH - 1:
                            nc.tensor.matmul(
                                ps_o[0 : HD + 1, :],
                                vext_tail4[c * 32 : c * 32 + tcsz, h, :],
                                es[par][sc],
                                start=False,
                                stop=True,
                                tile_position=(c * 32, 0),
                            )
                        else:
                            nc.tensor.matmul(
                                ps_o[0 : HD + 1, :],
                                vext_t[0 : CS[sc], sc, h, :],
                                es[par][sc],
                                start=(sc == 0),
                                stop=(sc == NCH - 1 and not packed_tail),
                            )
                    nc.vector.tensor_copy(
                        oe_big[:, par * IC + c, :], ps_o[0 : HD + 1, :]
                    )

            # denominator path: gather (1 DMA) -> reciprocal (1 DVE op) ->
            # DRAM bounce (1 store + 2 broadcast DMAs); the normalize muls
            # run early in the NEXT tile-iteration.
            nc.gpsimd.dma_start(den8_t[:], oe_big[HD : HD + 1, :, :])
            nc.vector.reciprocal(den8_t[:], den8_t[:])
            nc.gpsimd.dma_start(rden_dram[t], den8_t[:])
            ot_t = work.tile([P, IC, NT], mmdt, tag="ot")
            numer_t = work.tile([P, IC, NT], f32, tag="numer")
            den_t = work.tile([P, IC, NT], f32, tag="den")
            for par in (0, 1):
                nc.gpsimd.dma_start(
                    den_t[par * HD : (par + 1) * HD, :, :],
                    rden_dram[t, par : par + 1].to_broadcast((HD, IC, NT)),
                )
                nc.gpsimd.dma_start(
                    numer_t[par * HD : (par + 1) * HD, :, :],
                    oe_big[0:HD, par * IC : (par + 1) * IC, :],
                )

            # previous tile's output projection: fills the PE while tile t's
            # den DMAs fly.
            if pend is not None:
                emit_yproj(pend[0], pend[1])
            pend = (t, ot_t, numer_t, den_t)

        emit_muls(*pend)
        emit_yproj(pend[0], pend[1])

    _split_multi_waits(nc)
    return nc


_NC_CACHE: dict = {}


def _get_nc(flags):
    if flags not in _NC_CACHE:
        _NC_CACHE[flags] = _build_nc(*flags)
    return _NC_CACHE[flags]


def _prep_in_maps(x, context, context_mask, wq, bq, wkv, bkv, wp, bp, mmdt_name=None):
    if mmdt_name is None:
        mmdt_name = MMDT_NAME
    np_mm = _np_mm(getattr(mybir.dt, mmdt_name))
    cvt = lambda a: np.ascontiguousarray(a).astype(np_mm, copy=False)

    # --- context compaction: drop masked (True = padding) tokens ---------
    keep = [np.where(~context_mask[b])[0] for b in range(B)]
    cnts = [len(k) for k in keep]
    seff = max(16, -(-max(cnts) // 16) * 16)  # round up to mult of 16
    seff = min(seff, S)
    seff_pad = -(-seff // 128) * 128

    wqT = cvt(wq.T)
    wkT = cvt(wkv[:D].T)
    wvT = cvt(wkv[D:].T)
    wpT = cvt(wp.T)
    bq_c = np.ascontiguousarray(bq.reshape(D, 1), dtype=np.float32)
    bk_c = np.ascontiguousarray(bkv[:D].reshape(D, 1), dtype=np.float32)
    bv_r = cvt(bkv[D:].reshape(1, D))
    bp_r = cvt(bp.reshape(1, D))
    flags = (
        mmdt_name,
        int(seff),
        bool(np.any(bq != 0)),
        bool(np.any(bkv[:D] != 0)),
        bool(np.any(bkv[D:] != 0)),
        bool(np.any(bp != 0)),
    )
    in_maps = []
    for b in range(B):
        cnt = cnts[b]
        ctxc = np.zeros((D, seff), dtype=np_mm)
        ctxc[:, :cnt] = cvt(context[b][keep[b]].T)
        am = np.full((seff_pad, 1), np.float32(MASK_NEG), dtype=np.float32)
        am[:cnt] = 0.0
        in_maps.append(
            {
                "xT": cvt(x[b].T),
                "ctxT": ctxc,
                "wqT": wqT,
                "wkT": wkT,
                "wvT": wvT,
                "wpT": wpT,
                "bq": bq_c,
                "bk": bk_c,
                "bv": bv_r,
                "bp": bp_r,
                "amask": am,
            }
        )
    return in_maps, flags


def kernel(x, context, context_mask, wq, bq, wkv, bkv, wp, bp):
    from concourse.bass_utils import run_bass_kernel_spmd

    in_maps, flags = _prep_in_maps(
        x, context, context_mask, wq, bq, wkv, bkv, wp, bp
    )
    nc = _get_nc(flags)
    res = run_bass_kernel_spmd(nc, in_maps, list(range(B)))
    return np.stack([np.asarray(res.results[b]["y"]) for b in range(B)], axis=0)
